# revision 1
# baseline (speedup 1.0000x reference)
"""BoostedCausalAttention on 8 trn2 NeuronCores.

Sharding: core c -> (batch b = c//4, head-group g = c%4, 4 heads each).
Within a 4-core batch group (Megatron-style):
  - qkv projections + attention computed per head-group in "transposed"
    layout (feature on partitions, token on free axis), fp32r matmuls.
  - Per-head-pair AllGathers of pred^T (residual path) and corr^T; the
    gate MLP + output projection run on each core's 512-token strip, with
    the strip selected by a partition_id-derived dynamic DMA offset.
  - Final output: each core emits y[512 tokens, 1024] (pre-bias); host
    concatenates and adds bo.
"""

from contextlib import ExitStack

import numpy as np

import concourse.bass as bass
import concourse.bacc as bacc
import concourse.tile as tile
import concourse.mybir as mybir
from concourse import bass_utils

B, T, D = 2, 2048, 1024
H, DH = 16, 64
SCALE = DH ** -0.5
G = 4            # head groups (cores per batch)
HG = H // G      # heads per core = 4
CP = HG * DH     # channels per core = 256
TS = T // G      # token slice per core for gate/output phase = 512
KC = D // 128    # contraction chunks over D = 8
MASK_VAL = -30000.0

F32 = mybir.dt.float32
F32R = mybir.dt.float32r
ID = mybir.ActivationFunctionType.Identity
EXP = mybir.ActivationFunctionType.Exp
SIG = mybir.ActivationFunctionType.Sigmoid

GROUPS = [[0, 1, 2, 3], [4, 5, 6, 7]]


def _build(sim=False):
    nc = bacc.Bacc("TRN2", target_bir_lowering=False, debug=False, num_devices=8)

    xT = nc.dram_tensor("xT", [D, T], F32R, kind="ExternalInput")
    wqk0 = nc.dram_tensor("wqk0", [D, 2 * CP], F32R, kind="ExternalInput")
    wv0 = nc.dram_tensor("wv0", [D, CP], F32R, kind="ExternalInput")
    bqk0 = nc.dram_tensor("bqk0", [128, 4], F32, kind="ExternalInput")
    bv0 = nc.dram_tensor("bv0", [64, HG], F32, kind="ExternalInput")
    wqk1 = nc.dram_tensor("wqk1", [D, 2 * CP], F32R, kind="ExternalInput")
    wv1 = nc.dram_tensor("wv1", [D, CP], F32R, kind="ExternalInput")
    bqk1 = nc.dram_tensor("bqk1", [128, 4], F32, kind="ExternalInput")
    bv1 = nc.dram_tensor("bv1", [64, HG], F32, kind="ExternalInput")
    wg = nc.dram_tensor("wg", [2 * D, D], F32R, kind="ExternalInput")
    bg = nc.dram_tensor("bg", [128, D // 128], F32, kind="ExternalInput")
    wo = nc.dram_tensor("wo", [D, D], F32R, kind="ExternalInput")
    mask = nc.dram_tensor("mask", [128, 128], F32, kind="ExternalInput")
    onesc = nc.dram_tensor("onesc", [128, HG], F32R, kind="ExternalInput")
    ones64 = nc.dram_tensor("ones64", [1, 64], F32R, kind="ExternalInput")
    y = nc.dram_tensor("y", [TS, D], F32, kind="ExternalOutput")

    with tile.TileContext(nc) as tc, ExitStack() as glb:
        consts = glb.enter_context(tc.tile_pool(name="consts", bufs=1))
        # 8KB-per-partition slots: x^T/residual tiles, later reused by Wg
        big8 = glb.enter_context(tc.tile_pool(name="big8", bufs=8))
        dpool = glb.enter_context(tc.tile_pool(name="dpool", bufs=1, space="DRAM"))
        dsmall = glb.enter_context(tc.tile_pool(name="dsmall", bufs=4, space="DRAM"))

        # ---- constants ----
        bqk_sb = [consts.tile([128, 4], F32, name=f"bqk_sb{r}") for r in range(2)]
        bv_sb = [consts.tile([64, HG], F32, name=f"bv_sb{r}") for r in range(2)]
        bg_sb = consts.tile([128, D // 128], F32)
        mask_sb = consts.tile([128, 128], F32)
        ones_sb = consts.tile([128, HG], F32R)
        ones64_sb = consts.tile([1, 64], F32R)
        nc.gpsimd.dma_start(ones_sb[:], onesc[:, :])
        nc.gpsimd.dma_start(ones64_sb[:], ones64[:, :])
        nc.gpsimd.dma_start(bqk_sb[0][:], bqk0[:, :])
        nc.gpsimd.dma_start(bqk_sb[1][:], bqk1[:, :])
        nc.gpsimd.dma_start(bv_sb[0][:], bv0[:, :])
        nc.gpsimd.dma_start(bv_sb[1][:], bv1[:, :])
        nc.gpsimd.dma_start(bg_sb[:], bg[:, :])
        nc.gpsimd.dma_start(mask_sb[:], mask[:, :])

        # ---- x^T resident (becomes residual in place after round 0) ----
        xt = []
        for kc in range(KC):
            xt.append(big8.tile([128, T], F32R, name=f"xt{kc}", tag="b8"))

        # ---- collective DRAM buffers ----
        pred_part = dpool.tile([CP, T], F32R)          # round-0 AG input
        corr_part = dpool.tile([CP, T], F32R)          # round-1 AG input
        # AG outputs, split by head-pair so each half gathers as soon as
        # that pair's attention finishes: rows = 4 group members x 128
        pred_fh = [dpool.tile([4 * 128, T], F32R, name=f"pred_fh{i}")
                   for i in range(2)]
        corr_fh = [dpool.tile([4 * 128, T], F32R, name=f"corr_fh{i}")
                   for i in range(2)]

        wqk_pool = glb.enter_context(tc.tile_pool(name="wqk", bufs=KC))
        qk_pool = glb.enter_context(tc.tile_pool(name="qk", bufs=4))
        with ExitStack() as att:
            wv_pool = att.enter_context(tc.tile_pool(name="wv", bufs=KC))
            vaug_pool = att.enter_context(tc.tile_pool(name="vaug", bufs=16))
            p_pool = att.enter_context(tc.tile_pool(name="pp", bufs=4))
            o_pool = att.enter_context(tc.tile_pool(name="op", bufs=6))
            bc_pool = att.enter_context(tc.tile_pool(name="bc", bufs=2))
            sm_pool = att.enter_context(tc.tile_pool(name="sm", bufs=1))
            psum = att.enter_context(tc.tile_pool(name="psum", bufs=3, space="PSUM"))
            avps = att.enter_context(tc.tile_pool(name="avps", bufs=5, space="PSUM"))

            def load_wqk(wqk_d, rnd):
                wt = []
                for kc in range(KC):
                    t_ = wqk_pool.tile([128, 2 * CP], F32R,
                                       name=f"wqk{rnd}_{kc}", tag="wqk")
                    eng = nc.scalar if kc % 2 else nc.sync
                    eng.dma_start(t_[:], wqk_d[128 * kc:128 * (kc + 1), :])
                    wt.append(t_)
                return wt

            def load_wv(wv_d, rnd):
                wt = []
                for kc in range(KC):
                    t_ = wv_pool.tile([128, CP], F32R,
                                      name=f"wv{rnd}_{kc}", tag="wv")
                    eng = nc.scalar if kc % 2 else nc.sync
                    eng.dma_start(t_[:], wv_d[128 * kc:128 * (kc + 1), :])
                    wt.append(t_)
                return wt

            def proj_qk(wt, src, biasc, rnd):
                """q^T|k^T [512 rows, T] as 4 tiles [128, T] (0-1: q, 2-3: k)."""
                qk = []
                for jc in range(4):
                    qk.append(qk_pool.tile([128, T], F32R,
                                           name=f"qk{rnd}_{jc}", tag="qk"))
                for t4 in range(4):
                    for jc in range(4):
                        ps = psum.tile([128, 512], F32, tag="ps", name="ps_pqk")
                        for i, kc in enumerate([0, 2, 4, 6, 1, 3, 5, 7]):
                            nc.tensor.matmul(
                                ps[:], wt[kc][:, 128 * jc:128 * (jc + 1)],
                                src[kc][:, 512 * t4:512 * (t4 + 1)],
                                start=(i == 0), stop=(i == KC - 1))
                        nc.vector.tensor_scalar_add(
                            qk[jc][:, 512 * t4:512 * (t4 + 1)], ps[:],
                            biasc[:, jc:jc + 1])
                return qk

            def proj_v(wt, src, rnd):
                """v in natural layout + ones col: 16 tiles [128, HG, DH+1]."""
                va = []
                for tb in range(16):
                    t_ = vaug_pool.tile([128, HG, DH + 1], F32R,
                                        name=f"va{rnd}_{tb}", tag="va")
                    ps = psum.tile([128, 512], F32, tag="ps", name="ps_pv")
                    for i, kc in enumerate([0, 2, 4, 6, 1, 3, 5, 7]):
                        nc.tensor.matmul(
                            ps[:, 0:CP], src[kc][:, 128 * tb:128 * (tb + 1)],
                            wt[kc][:], start=(i == 0), stop=(i == KC - 1))
                    nc.vector.tensor_copy(
                        t_[:, :, 0:DH],
                        ps[:, 0:CP].rearrange("p (h d) -> p h d", h=HG))
                    nc.vector.tensor_copy(t_[:, :, DH:DH + 1],
                                          ones_sb[:, :, None])
                    va.append(t_)
                return va

            def attend(qk, va, biasv, part_dst, hh_done=None):
                """Causal attention for 4 heads; writes normalized pred^T
                parts straight to DRAM (AG layout and/or A2A layout)."""
                for hh in range(2):      # head pairs share PE via row groups
                    ob = [o_pool.tile([64, T], F32R, tag="o", name=f"o{h2}")
                          for h2 in range(2)]
                    for q4 in range(4):
                        nblk = 4 * (q4 + 1)
                        av = [avps.tile([DH + 1, 512], F32, tag="av",
                                        name=f"av{h2}") for h2 in range(2)]
                        for kb in range(nblk):
                            for h2 in range(2):
                                h = 2 * hh + h2
                                base = 64 * h2
                                qt, kt = qk[hh], qk[2 + hh]
                                diag = kb - 4 * q4
                                c0 = max(0, 128 * diag)
                                npr = 512 - c0
                                ps = psum.tile([128, 512], F32, tag="ps",
                                               name=f"s{h2}")
                                nc.tensor.matmul(
                                    ps[:, 0:npr],
                                    kt[base:base + 64,
                                       128 * kb:128 * (kb + 1)],
                                    qt[base:base + 64,
                                       512 * q4 + c0:512 * (q4 + 1)],
                                    start=True, stop=True)
                                if diag >= 0:
                                    nc.vector.tensor_add(
                                        ps[:, 0:128], ps[:, 0:128], mask_sb[:])
                                p = p_pool.tile([128, 512], F32R, tag="p",
                                                name=f"p{h2}")
                                nc.scalar.activation(p[:, 0:npr], ps[:, 0:npr],
                                                     EXP, scale=SCALE)
                                nc.tensor.matmul(
                                    av[h2][:, c0:512], va[kb][:, h, :],
                                    p[:, 0:npr],
                                    start=(kb == 0), stop=(kb == nblk - 1))
                        recr = sm_pool.tile([1, 1024], F32R, tag="recr",
                                            name="recr")
                        with nc.allow_low_precision(
                                reason="softmax recip rounds to f32r"):
                            nc.vector.reciprocal(recr[0:1, 0:512],
                                                 av[0][DH:DH + 1, :])
                            nc.vector.reciprocal(recr[0:1, 512:1024],
                                                 av[1][DH:DH + 1, :])
                        for h2 in range(2):
                            h = 2 * hh + h2
                            bps = avps.tile([65, 512], F32, tag="av",
                                            name="bps")
                            nc.tensor.matmul(
                                bps[0:64, :], ones64_sb[:],
                                recr[0:1, 512 * h2:512 * (h2 + 1)],
                                start=True, stop=True)
                            bc = bc_pool.tile([64, 512], F32, tag="bc",
                                              name="bc")
                            nc.vector.tensor_copy(bc[:], bps[0:64, :])
                            osl = ob[h2][:, 512 * q4:512 * (q4 + 1)]
                            nc.vector.tensor_mul(osl, av[h2][0:DH, :], bc[:])
                            nc.vector.tensor_scalar_add(osl, osl,
                                                        biasv[:, h:h + 1])
                    for h2 in range(2):
                        h = 2 * hh + h2
                        nc.scalar.dma_start(
                            part_dst[64 * h:64 * (h + 1), :], ob[h2][:])
                    if hh_done is not None:
                        hh_done(hh)

            # ================= round 0 =================
            wv0_t = load_wv(wv0, 0)
            for hf in range(4):
                for kc in range(KC):
                    eng = nc.scalar if (kc + hf) % 2 else nc.sync
                    eng.dma_start(xt[kc][:, 512 * hf:512 * (hf + 1)],
                                  xT[128 * kc:128 * (kc + 1),
                                     512 * hf:512 * (hf + 1)])
                if hf == 0:
                    wqk0_t = load_wqk(wqk0, 0)
            va0 = proj_v(wv0_t, xt, 0)
            qk0 = proj_qk(wqk0_t, xt, bqk_sb[0], 0)
            def ag_pred(hh):
                if sim:
                    nc.sync.dma_start(pred_fh[hh][0:128, 0:64],
                                      pred_part[128 * hh:128 * (hh + 1), 0:64])
                else:
                    nc.gpsimd.collective_compute(
                        "AllGather", mybir.AluOpType.bypass,
                        replica_groups=GROUPS,
                        ins=[pred_part[128 * hh:128 * (hh + 1), :]],
                        outs=[pred_fh[hh][:, :]])

            attend(qk0, va0, bv_sb[0], pred_part, hh_done=ag_pred)

            # residual in place: xt <- xt - pred^T (column halves so the
            # t4-major round-1 projection unblocks after the first half)
            for hf in range(2):
                cs = slice(1024 * hf, 1024 * (hf + 1))
                for kc in [0, 2, 4, 6, 1, 3, 5, 7]:
                    pt = o_pool.tile([128, 1024], F32R, tag="o", name="predld")
                    src_t = pred_fh[kc % 2]
                    r0 = 128 * (kc // 2)
                    nc.sync.dma_start(pt[:], src_t[r0:r0 + 128, cs])
                    nc.vector.tensor_sub(xt[kc][:, cs], xt[kc][:, cs], pt[:])

            # ================= round 1 =================
            wv1_t = load_wv(wv1, 1)
            wqk1_t = load_wqk(wqk1, 1)
            va1 = proj_v(wv1_t, xt, 1)
            qk1 = proj_qk(wqk1_t, xt, bqk_sb[1], 1)

            # prefetch gate-phase pred slices into the wqk slots that free
            # up after the round-1 projection (overlaps attention 1)
            pid = nc.scalar.partition_id()
            greg = nc.scalar.alloc_register("gslice")
            nc.scalar.reg_mod(greg, pid, G)
            gsel = nc.scalar.snap(greg, donate=True, min_val=0, max_val=G - 1)
            pred_vh = [t[:, :].rearrange("d (s t) -> d s t", s=G)
                       for t in pred_fh]
            predg = []
            for cc in range(KC):
                pg_ = wqk_pool.tile([128, TS], F32R, name=f"predg{cc}",
                                    tag="wqk")
                r0 = 128 * (cc // 2)
                nc.scalar.dma_start(
                    pg_[:], pred_vh[cc % 2][r0:r0 + 128,
                                            bass.ds(gsel, 1), :].squeeze(1))
                predg.append(pg_)

            def ag_corr(hh):
                if sim:
                    nc.sync.dma_start(corr_fh[hh][0:128, 0:64],
                                      corr_part[128 * hh:128 * (hh + 1), 0:64])
                else:
                    nc.gpsimd.collective_compute(
                        "AllGather", mybir.AluOpType.bypass,
                        replica_groups=GROUPS,
                        ins=[corr_part[128 * hh:128 * (hh + 1), :]],
                        outs=[corr_fh[hh][:, :]])

            attend(qk1, va1, bv_sb[1], corr_part, hh_done=ag_corr)

        # ================= gate + output ==================
        with ExitStack() as gat:
            gp_pool = gat.enter_context(tc.tile_pool(name="gp", bufs=8))
            wo_pool = gat.enter_context(tc.tile_pool(name="wo", bufs=KC))
            y_pool = gat.enter_context(tc.tile_pool(name="yp", bufs=2))
            ps2 = gat.enter_context(tc.tile_pool(name="ps2", bufs=8,
                                                 space="PSUM"))

            # Wg reuses the 8KB big8 slots freed by x^T/residual:
            # tile i holds contraction chunks 2i (cols 0:1024) and 2i+1.
            wg_t = []
            for i in range(KC):
                t_ = big8.tile([128, 2 * D], F32R, name=f"wg{i}", tag="b8")
                nc.scalar.dma_start(
                    t_[:].rearrange("p (a d) -> p a d", a=2),
                    bass.AP(tensor=wg, offset=256 * i * D,
                            ap=[[D, 128], [128 * D, 2], [1, D]]))
                wg_t.append(t_)

            def wg_slice(cc, jc):
                return wg_t[cc // 2][:, D * (cc % 2) + 128 * jc:
                                     D * (cc % 2) + 128 * (jc + 1)]

            wo_t = []
            for cc in range(KC):
                t_ = wo_pool.tile([128, D], F32R, name=f"wo{cc}", tag="wo")
                nc.sync.dma_start(t_[:], wo[128 * cc:128 * (cc + 1), :])
                wo_t.append(t_)

            corr_vh = [t[:, :].rearrange("d (s t) -> d s t", s=G)
                       for t in corr_fh]
            corr_t = [qk_pool.tile([128, T], F32R, name=f"corrt{i}", tag="qk")
                      for i in range(2)]
            for cc in range(KC):
                r0 = 128 * (cc // 2)
                nc.scalar.dma_start(
                    corr_t[cc // 4][:, 512 * (cc % 4):512 * (cc % 4 + 1)],
                    corr_vh[cc % 2][r0:r0 + 128,
                                    bass.ds(gsel, 1), :].squeeze(1))
            corrg = [corr_t[cc // 4][:, 512 * (cc % 4):512 * (cc % 4 + 1)]
                     for cc in range(KC)]

            gps = []
            for jc in range(KC):
                ps = ps2.tile([128, 512], F32, tag="ps2", name=f"ps_g{jc}")
                for cc in range(KC):
                    nc.tensor.matmul(ps[:], wg_slice(cc, jc), predg[cc][:],
                                     start=(cc == 0), stop=False)
                gps.append(ps)
            pgt = []
            for jc in range(KC):
                ps = gps[jc]
                for cc in range(KC, 16):
                    nc.tensor.matmul(ps[:], wg_slice(cc, jc), corrg[cc - KC],
                                     start=False, stop=(cc == 15))
                gt = gp_pool.tile([128, TS], F32R, name=f"gate{jc}", tag="gp")
                nc.scalar.activation(gt[:], ps[:], SIG, bias=bg_sb[:, jc:jc + 1])
                nc.vector.tensor_mul(gt[:], gt[:], corrg[jc])
                nc.vector.tensor_add(gt[:], gt[:], predg[jc][:])
                pgt.append(gt)

            for tb in range(4):
                yt = y_pool.tile([128, D], F32, tag="y", name="yt")
                for n2 in range(2):
                    ps = ps2.tile([128, 512], F32, tag="ps2", name="ps_y")
                    for cc in range(KC):
                        nc.tensor.matmul(
                            ps[:], pgt[cc][:, 128 * tb:128 * (tb + 1)],
                            wo_t[cc][:, 512 * n2:512 * (n2 + 1)],
                            start=(cc == 0), stop=(cc == KC - 1))
                    nc.vector.tensor_copy(yt[:, 512 * n2:512 * (n2 + 1)],
                                          ps[:])
                nc.sync.dma_start(y[128 * tb:128 * (tb + 1), :], yt[:])

    nc.compile()
    return nc


_NC = None


def _get_nc():
    global _NC
    if _NC is None:
        _NC = _build()
    return _NC


def make_in_maps(x, Wqkv0, bqkv0, Wqkv1, bqkv1, Wg, bg, Wo, bo):
    mask_np = np.where(np.arange(128)[:, None] > np.arange(128)[None, :],
                       np.float32(MASK_VAL), np.float32(0.0)).astype(np.float32)
    ones_np = np.ones((128, HG), np.float32)
    bg_a = np.ascontiguousarray(bg.reshape(D // 128, 128).T.astype(np.float32))
    wg_np = np.ascontiguousarray(Wg.astype(np.float32))
    wo_np = np.ascontiguousarray(Wo.astype(np.float32))

    in_maps = []
    for c in range(8):
        b, g = divmod(c, G)
        cq = slice(CP * g, CP * (g + 1))
        ck = slice(D + CP * g, D + CP * (g + 1))
        cv = slice(2 * D + CP * g, 2 * D + CP * (g + 1))
        m = {
            "xT": np.ascontiguousarray(x[b].T.astype(np.float32)),
            "mask": mask_np, "onesc": ones_np, "bg": bg_a,
            "ones64": np.ones((1, 64), np.float32),
            "wg": wg_np, "wo": wo_np,
        }
        for r, (W, bb) in enumerate(((Wqkv0, bqkv0), (Wqkv1, bqkv1))):
            m[f"wqk{r}"] = np.ascontiguousarray(
                np.concatenate([W[:, cq], W[:, ck]], axis=1).astype(np.float32))
            m[f"wv{r}"] = np.ascontiguousarray(W[:, cv].astype(np.float32))
            bqk_cat = np.concatenate([bb[cq], bb[ck]]).astype(np.float32)
            m[f"bqk{r}"] = np.ascontiguousarray(bqk_cat.reshape(4, 128).T)
            m[f"bv{r}"] = np.ascontiguousarray(
                bb[cv].astype(np.float32).reshape(HG, 64).T)
        in_maps.append(m)
    return in_maps


def assemble(results, bo):
    out = np.empty((B, T, D), np.float32)
    for c in range(8):
        b, g = divmod(c, G)
        out[b, TS * g:TS * (g + 1), :] = results[c]["y"]
    return out + bo.astype(np.float32)


def kernel(x, Wqkv0, bqkv0, Wqkv1, bqkv1, Wg, bg, Wo, bo):
    args = [np.asarray(a) for a in
            (x, Wqkv0, bqkv0, Wqkv1, bqkv1, Wg, bg, Wo, bo)]
    nc = _get_nc()
    in_maps = make_in_maps(*args)
    res = bass_utils.run_bass_kernel_spmd(nc, in_maps, core_ids=list(range(8)))
    return assemble(res.results, args[8])



# revision 4
# speedup vs baseline: 44.8296x; 44.8296x over previous
"""BoostedCausalAttention on 8 trn2 NeuronCores.

Sharding: core c -> (batch b = c//4, head-group g = c%4, 4 heads each).
Within a 4-core batch group (Megatron-style):
  - qkv projections + attention computed per head-group in "transposed"
    layout (feature on partitions, token on free axis), fp32r matmuls.
  - Per-head-pair AllGathers of pred^T (residual path) and corr^T; the
    gate MLP + output projection run on each core's 512-token strip, with
    the strip selected by a partition_id-derived dynamic DMA offset.
  - Final output: each core emits y[512 tokens, 1024] (pre-bias); host
    concatenates and adds bo.
"""

from contextlib import ExitStack

import numpy as np

import concourse.bass as bass
import concourse.bacc as bacc
import concourse.tile as tile
import concourse.mybir as mybir
from concourse import bass_utils

B, T, D = 2, 2048, 1024
H, DH = 16, 64
SCALE = DH ** -0.5
G = 4            # head groups (cores per batch)
HG = H // G      # heads per core = 4
CP = HG * DH     # channels per core = 256
TS = T // G      # token slice per core for gate/output phase = 512
KC = D // 128    # contraction chunks over D = 8
MASK_VAL = -30000.0

F32 = mybir.dt.float32
F32R = mybir.dt.float32r
ID = mybir.ActivationFunctionType.Identity
EXP = mybir.ActivationFunctionType.Exp
SIG = mybir.ActivationFunctionType.Sigmoid

GROUPS = [[0, 1, 2, 3], [4, 5, 6, 7]]


def _build(sim=False, repeat=1):
    nc = bacc.Bacc("TRN2", target_bir_lowering=False, debug=False, num_devices=8)

    xT = nc.dram_tensor("xT", [D, T], F32R, kind="ExternalInput")
    wqk0 = nc.dram_tensor("wqk0", [D, 2 * CP], F32R, kind="ExternalInput")
    wv0 = nc.dram_tensor("wv0", [D, CP], F32R, kind="ExternalInput")
    bqk0 = nc.dram_tensor("bqk0", [128, 4], F32, kind="ExternalInput")
    bv0 = nc.dram_tensor("bv0", [64, HG], F32, kind="ExternalInput")
    wqk1 = nc.dram_tensor("wqk1", [D, 2 * CP], F32R, kind="ExternalInput")
    wv1 = nc.dram_tensor("wv1", [D, CP], F32R, kind="ExternalInput")
    bqk1 = nc.dram_tensor("bqk1", [128, 4], F32, kind="ExternalInput")
    bv1 = nc.dram_tensor("bv1", [64, HG], F32, kind="ExternalInput")
    wg = nc.dram_tensor("wg", [2 * D, D], F32R, kind="ExternalInput")
    bg = nc.dram_tensor("bg", [128, D // 128], F32, kind="ExternalInput")
    wo = nc.dram_tensor("wo", [D, D], F32R, kind="ExternalInput")
    mask = nc.dram_tensor("mask", [128, 128], F32, kind="ExternalInput")
    onesc = nc.dram_tensor("onesc", [128, HG], F32R, kind="ExternalInput")
    ones64 = nc.dram_tensor("ones64", [1, 64], F32R, kind="ExternalInput")
    y = nc.dram_tensor("y", [TS, D], F32, kind="ExternalOutput")

    with tile.TileContext(nc) as tc:
        for _rep in range(repeat):
            _build_body(nc, tc, sim, _rep,
                        xT, wqk0, wv0, bqk0, bv0, wqk1, wv1, bqk1, bv1,
                        wg, bg, wo, mask, onesc, ones64, y)

    nc.compile()
    return nc


def _build_body(nc, tc, sim, _rep,
                xT, wqk0, wv0, bqk0, bv0, wqk1, wv1, bqk1, bv1,
                wg, bg, wo, mask, onesc, ones64, y):
    with ExitStack() as glb:
        consts = glb.enter_context(tc.tile_pool(name="consts", bufs=1))
        # 8KB-per-partition slots: x^T/residual tiles, later reused by Wg
        big8 = glb.enter_context(tc.tile_pool(name="big8", bufs=8))
        dpool = glb.enter_context(tc.tile_pool(name="dpool", bufs=1, space="DRAM"))
        dsmall = glb.enter_context(tc.tile_pool(name="dsmall", bufs=4, space="DRAM"))

        # ---- constants ----
        bqk_sb = [consts.tile([128, 4], F32, name=f"bqk_sb{r}") for r in range(2)]
        bv_sb = [consts.tile([64, HG], F32, name=f"bv_sb{r}") for r in range(2)]
        bg_sb = consts.tile([128, D // 128], F32)
        mask_sb = consts.tile([128, 128], F32)
        ones_sb = consts.tile([128, HG], F32R)
        ones64_sb = consts.tile([1, 64], F32R)
        nc.gpsimd.dma_start(ones_sb[:], onesc[:, :])
        nc.gpsimd.dma_start(ones64_sb[:], ones64[:, :])
        nc.gpsimd.dma_start(bqk_sb[0][:], bqk0[:, :])
        nc.gpsimd.dma_start(bqk_sb[1][:], bqk1[:, :])
        nc.gpsimd.dma_start(bv_sb[0][:], bv0[:, :])
        nc.gpsimd.dma_start(bv_sb[1][:], bv1[:, :])
        nc.gpsimd.dma_start(bg_sb[:], bg[:, :])
        nc.gpsimd.dma_start(mask_sb[:], mask[:, :])

        # ---- x^T resident (becomes residual in place after round 0) ----
        xt = []
        for kc in range(KC):
            xt.append(big8.tile([128, T], F32R, name=f"xt{kc}", tag="b8"))

        # ---- collective DRAM buffers ----
        pred_part = dpool.tile([CP, T], F32R)          # round-0 AG input
        corr_part = dpool.tile([CP, T], F32R)          # round-1 AG input
        # AG outputs, split by head-pair so each half gathers as soon as
        # that pair's attention finishes: rows = 4 group members x 128
        pred_fh = [dpool.tile([4 * 128, T], F32R, name=f"pred_fh{i}")
                   for i in range(2)]
        corr_fh = [dpool.tile([4 * 128, T], F32R, name=f"corr_fh{i}")
                   for i in range(2)]

        wqk_pool = glb.enter_context(tc.tile_pool(name="wqk", bufs=KC))
        qk_pool = glb.enter_context(tc.tile_pool(name="qk", bufs=4))
        with ExitStack() as att:
            wv_pool = att.enter_context(tc.tile_pool(name="wv", bufs=KC))
            vaug_pool = att.enter_context(tc.tile_pool(name="vaug", bufs=16))
            p_pool = att.enter_context(tc.tile_pool(name="pp", bufs=4))
            o_pool = att.enter_context(tc.tile_pool(name="op", bufs=6))
            bc_pool = att.enter_context(tc.tile_pool(name="bc", bufs=2))
            sm_pool = att.enter_context(tc.tile_pool(name="sm", bufs=1))
            psum = att.enter_context(tc.tile_pool(name="psum", bufs=3, space="PSUM"))
            avps = att.enter_context(tc.tile_pool(name="avps", bufs=5, space="PSUM"))

            def load_wqk(wqk_d, rnd):
                wt = []
                for kc in range(KC):
                    t_ = wqk_pool.tile([128, 2 * CP], F32R,
                                       name=f"wqk{rnd}_{kc}", tag="wqk")
                    eng = nc.scalar if kc % 2 else nc.sync
                    eng.dma_start(t_[:], wqk_d[128 * kc:128 * (kc + 1), :])
                    wt.append(t_)
                return wt

            def load_wv(wv_d, rnd):
                wt = []
                for kc in range(KC):
                    t_ = wv_pool.tile([128, CP], F32R,
                                      name=f"wv{rnd}_{kc}", tag="wv")
                    eng = nc.scalar if kc % 2 else nc.sync
                    eng.dma_start(t_[:], wv_d[128 * kc:128 * (kc + 1), :])
                    wt.append(t_)
                return wt

            def proj_qk(wt, src, biasc, rnd):
                """q^T|k^T [512 rows, T] as 4 tiles [128, T] (0-1: q, 2-3: k)."""
                qk = []
                for jc in range(4):
                    qk.append(qk_pool.tile([128, T], F32R,
                                           name=f"qk{rnd}_{jc}", tag="qk"))
                for t4 in range(4):
                    for jc in range(4):
                        ps = psum.tile([128, 512], F32, tag="ps", name="ps_pqk")
                        for i, kc in enumerate([0, 2, 4, 6, 1, 3, 5, 7]):
                            nc.tensor.matmul(
                                ps[:], wt[kc][:, 128 * jc:128 * (jc + 1)],
                                src[kc][:, 512 * t4:512 * (t4 + 1)],
                                start=(i == 0), stop=(i == KC - 1))
                        nc.vector.tensor_scalar_add(
                            qk[jc][:, 512 * t4:512 * (t4 + 1)], ps[:],
                            biasc[:, jc:jc + 1])
                return qk

            def proj_v(wt, src, rnd):
                """v in natural layout + ones col: 16 tiles [128, HG, DH+1]."""
                va = []
                for tb in range(16):
                    t_ = vaug_pool.tile([128, HG, DH + 1], F32R,
                                        name=f"va{rnd}_{tb}", tag="va")
                    ps = psum.tile([128, 512], F32, tag="ps", name="ps_pv")
                    for i, kc in enumerate([0, 2, 4, 6, 1, 3, 5, 7]):
                        nc.tensor.matmul(
                            ps[:, 0:CP], src[kc][:, 128 * tb:128 * (tb + 1)],
                            wt[kc][:], start=(i == 0), stop=(i == KC - 1))
                    nc.vector.tensor_copy(
                        t_[:, :, 0:DH],
                        ps[:, 0:CP].rearrange("p (h d) -> p h d", h=HG))
                    nc.vector.tensor_copy(t_[:, :, DH:DH + 1],
                                          ones_sb[:, :, None])
                    va.append(t_)
                return va

            def attend(qk, va, biasv, part_dst, hh_done=None):
                """Causal attention for 4 heads; writes normalized pred^T
                parts straight to DRAM (AG layout and/or A2A layout)."""
                for hh in range(2):      # head pairs share PE via row groups
                    ob = [o_pool.tile([64, T], F32R, tag="o", name=f"o{h2}")
                          for h2 in range(2)]
                    for q4 in range(4):
                        nblk = 4 * (q4 + 1)
                        av = [avps.tile([DH + 1, 512], F32, tag="av",
                                        name=f"av{h2}") for h2 in range(2)]
                        for kb in range(nblk):
                            for h2 in range(2):
                                h = 2 * hh + h2
                                base = 64 * h2
                                qt, kt = qk[hh], qk[2 + hh]
                                diag = kb - 4 * q4
                                c0 = max(0, 128 * diag)
                                npr = 512 - c0
                                ps = psum.tile([128, 512], F32, tag="ps",
                                               name=f"s{h2}")
                                nc.tensor.matmul(
                                    ps[:, 0:npr],
                                    kt[base:base + 64,
                                       128 * kb:128 * (kb + 1)],
                                    qt[base:base + 64,
                                       512 * q4 + c0:512 * (q4 + 1)],
                                    start=True, stop=True)
                                if diag >= 0:
                                    nc.vector.tensor_add(
                                        ps[:, 0:128], ps[:, 0:128], mask_sb[:])
                                p = p_pool.tile([128, 512], F32R, tag="p",
                                                name=f"p{h2}")
                                nc.scalar.activation(p[:, 0:npr], ps[:, 0:npr],
                                                     EXP, scale=SCALE)
                                nc.tensor.matmul(
                                    av[h2][:, c0:512], va[kb][:, h, :],
                                    p[:, 0:npr],
                                    start=(kb == 0), stop=(kb == nblk - 1))
                        recr = sm_pool.tile([1, 1024], F32R, tag="recr",
                                            name="recr")
                        with nc.allow_low_precision(
                                reason="softmax recip rounds to f32r"):
                            nc.vector.reciprocal(recr[0:1, 0:512],
                                                 av[0][DH:DH + 1, :])
                            nc.vector.reciprocal(recr[0:1, 512:1024],
                                                 av[1][DH:DH + 1, :])
                        for h2 in range(2):
                            h = 2 * hh + h2
                            bps = avps.tile([65, 512], F32, tag="av",
                                            name="bps")
                            nc.tensor.matmul(
                                bps[0:64, :], ones64_sb[:],
                                recr[0:1, 512 * h2:512 * (h2 + 1)],
                                start=True, stop=True)
                            bc = bc_pool.tile([64, 512], F32, tag="bc",
                                              name="bc")
                            nc.vector.tensor_copy(bc[:], bps[0:64, :])
                            osl = ob[h2][:, 512 * q4:512 * (q4 + 1)]
                            nc.vector.tensor_mul(osl, av[h2][0:DH, :], bc[:])
                            nc.vector.tensor_scalar_add(osl, osl,
                                                        biasv[:, h:h + 1])
                    for h2 in range(2):
                        h = 2 * hh + h2
                        nc.scalar.dma_start(
                            part_dst[64 * h:64 * (h + 1), :], ob[h2][:])
                    if hh_done is not None:
                        hh_done(hh)

            # ================= round 0 =================
            wv0_t = load_wv(wv0, 0)
            for hf in range(4):
                for kc in range(KC):
                    eng = nc.scalar if (kc + hf) % 2 else nc.sync
                    eng.dma_start(xt[kc][:, 512 * hf:512 * (hf + 1)],
                                  xT[128 * kc:128 * (kc + 1),
                                     512 * hf:512 * (hf + 1)])
                if hf == 0:
                    wqk0_t = load_wqk(wqk0, 0)
            va0 = proj_v(wv0_t, xt, 0)
            qk0 = proj_qk(wqk0_t, xt, bqk_sb[0], 0)
            def ag_pred(hh):
                if sim:
                    nc.sync.dma_start(pred_fh[hh][0:128, 0:64],
                                      pred_part[128 * hh:128 * (hh + 1), 0:64])
                else:
                    nc.gpsimd.collective_compute(
                        "AllGather", mybir.AluOpType.bypass,
                        replica_groups=GROUPS,
                        ins=[pred_part[128 * hh:128 * (hh + 1), :]],
                        outs=[pred_fh[hh][:, :]])

            attend(qk0, va0, bv_sb[0], pred_part, hh_done=ag_pred)

            # residual in place: xt <- xt - pred^T (column halves so the
            # t4-major round-1 projection unblocks after the first half)
            for hf in range(2):
                cs = slice(1024 * hf, 1024 * (hf + 1))
                for kc in [0, 2, 4, 6, 1, 3, 5, 7]:
                    pt = o_pool.tile([128, 1024], F32R, tag="o", name="predld")
                    src_t = pred_fh[kc % 2]
                    r0 = 128 * (kc // 2)
                    nc.sync.dma_start(pt[:], src_t[r0:r0 + 128, cs])
                    nc.vector.tensor_sub(xt[kc][:, cs], xt[kc][:, cs], pt[:])

            # ================= round 1 =================
            wv1_t = load_wv(wv1, 1)
            wqk1_t = load_wqk(wqk1, 1)
            va1 = proj_v(wv1_t, xt, 1)
            qk1 = proj_qk(wqk1_t, xt, bqk_sb[1], 1)

            # prefetch gate-phase pred slices into the wqk slots that free
            # up after the round-1 projection (overlaps attention 1)
            pid = nc.scalar.partition_id()
            greg = nc.scalar.alloc_register(f"gslice{_rep}")
            nc.scalar.reg_mod(greg, pid, G)
            gsel = nc.scalar.snap(greg, donate=True, min_val=0, max_val=G - 1)
            pred_vh = [t[:, :].rearrange("d (s t) -> d s t", s=G)
                       for t in pred_fh]
            predg = []
            for cc in range(KC):
                pg_ = wqk_pool.tile([128, TS], F32R, name=f"predg{cc}",
                                    tag="wqk")
                r0 = 128 * (cc // 2)
                nc.scalar.dma_start(
                    pg_[:], pred_vh[cc % 2][r0:r0 + 128,
                                            bass.ds(gsel, 1), :].squeeze(1))
                predg.append(pg_)

            def ag_corr(hh):
                if sim:
                    nc.sync.dma_start(corr_fh[hh][0:128, 0:64],
                                      corr_part[128 * hh:128 * (hh + 1), 0:64])
                else:
                    nc.gpsimd.collective_compute(
                        "AllGather", mybir.AluOpType.bypass,
                        replica_groups=GROUPS,
                        ins=[corr_part[128 * hh:128 * (hh + 1), :]],
                        outs=[corr_fh[hh][:, :]])

            attend(qk1, va1, bv_sb[1], corr_part, hh_done=ag_corr)

        # ================= gate + output ==================
        with ExitStack() as gat:
            gp_pool = gat.enter_context(tc.tile_pool(name="gp", bufs=8))
            wo_pool = gat.enter_context(tc.tile_pool(name="wo", bufs=KC))
            y_pool = gat.enter_context(tc.tile_pool(name="yp", bufs=2))
            ps2 = gat.enter_context(tc.tile_pool(name="ps2", bufs=8,
                                                 space="PSUM"))

            # Wg reuses the 8KB big8 slots freed by x^T/residual:
            # tile i holds contraction chunks 2i (cols 0:1024) and 2i+1.
            wg_t = []
            for i in range(KC):
                t_ = big8.tile([128, 2 * D], F32R, name=f"wg{i}", tag="b8")
                nc.scalar.dma_start(
                    t_[:].rearrange("p (a d) -> p a d", a=2),
                    bass.AP(tensor=wg, offset=256 * i * D,
                            ap=[[D, 128], [128 * D, 2], [1, D]]))
                wg_t.append(t_)

            def wg_slice(cc, jc):
                return wg_t[cc // 2][:, D * (cc % 2) + 128 * jc:
                                     D * (cc % 2) + 128 * (jc + 1)]

            wo_t = []
            for cc in range(KC):
                t_ = wo_pool.tile([128, D], F32R, name=f"wo{cc}", tag="wo")
                nc.sync.dma_start(t_[:], wo[128 * cc:128 * (cc + 1), :])
                wo_t.append(t_)

            corr_vh = [t[:, :].rearrange("d (s t) -> d s t", s=G)
                       for t in corr_fh]
            corr_t = [qk_pool.tile([128, T], F32R, name=f"corrt{i}", tag="qk")
                      for i in range(2)]
            for cc in range(KC):
                r0 = 128 * (cc // 2)
                nc.scalar.dma_start(
                    corr_t[cc // 4][:, 512 * (cc % 4):512 * (cc % 4 + 1)],
                    corr_vh[cc % 2][r0:r0 + 128,
                                    bass.ds(gsel, 1), :].squeeze(1))
            corrg = [corr_t[cc // 4][:, 512 * (cc % 4):512 * (cc % 4 + 1)]
                     for cc in range(KC)]

            gps = []
            for jc in range(KC):
                ps = ps2.tile([128, 512], F32, tag="ps2", name=f"ps_g{jc}")
                for cc in range(KC):
                    nc.tensor.matmul(ps[:], wg_slice(cc, jc), predg[cc][:],
                                     start=(cc == 0), stop=False)
                gps.append(ps)
            pgt = []
            for jc in range(KC):
                ps = gps[jc]
                for cc in range(KC, 16):
                    nc.tensor.matmul(ps[:], wg_slice(cc, jc), corrg[cc - KC],
                                     start=False, stop=(cc == 15))
                gt = gp_pool.tile([128, TS], F32R, name=f"gate{jc}", tag="gp")
                nc.scalar.activation(gt[:], ps[:], SIG, bias=bg_sb[:, jc:jc + 1])
                nc.vector.tensor_mul(gt[:], gt[:], corrg[jc])
                nc.vector.tensor_add(gt[:], gt[:], predg[jc][:])
                pgt.append(gt)

            for tb in range(4):
                yt = y_pool.tile([128, D], F32, tag="y", name="yt")
                for n2 in range(2):
                    ps = ps2.tile([128, 512], F32, tag="ps2", name="ps_y")
                    for cc in range(KC):
                        nc.tensor.matmul(
                            ps[:], pgt[cc][:, 128 * tb:128 * (tb + 1)],
                            wo_t[cc][:, 512 * n2:512 * (n2 + 1)],
                            start=(cc == 0), stop=(cc == KC - 1))
                    nc.vector.tensor_copy(yt[:, 512 * n2:512 * (n2 + 1)],
                                          ps[:])
                nc.sync.dma_start(y[128 * tb:128 * (tb + 1), :], yt[:])


_NC = None


def _get_nc():
    global _NC
    if _NC is None:
        _NC = _build()
    return _NC


def make_in_maps(x, Wqkv0, bqkv0, Wqkv1, bqkv1, Wg, bg, Wo, bo):
    mask_np = np.where(np.arange(128)[:, None] > np.arange(128)[None, :],
                       np.float32(MASK_VAL), np.float32(0.0)).astype(np.float32)
    ones_np = np.ones((128, HG), np.float32)
    bg_a = np.ascontiguousarray(bg.reshape(D // 128, 128).T.astype(np.float32))
    wg_np = np.ascontiguousarray(Wg.astype(np.float32))
    wo_np = np.ascontiguousarray(Wo.astype(np.float32))

    in_maps = []
    for c in range(8):
        b, g = divmod(c, G)
        cq = slice(CP * g, CP * (g + 1))
        ck = slice(D + CP * g, D + CP * (g + 1))
        cv = slice(2 * D + CP * g, 2 * D + CP * (g + 1))
        m = {
            "xT": np.ascontiguousarray(x[b].T.astype(np.float32)),
            "mask": mask_np, "onesc": ones_np, "bg": bg_a,
            "ones64": np.ones((1, 64), np.float32),
            "wg": wg_np, "wo": wo_np,
        }
        for r, (W, bb) in enumerate(((Wqkv0, bqkv0), (Wqkv1, bqkv1))):
            m[f"wqk{r}"] = np.ascontiguousarray(
                np.concatenate([W[:, cq], W[:, ck]], axis=1).astype(np.float32))
            m[f"wv{r}"] = np.ascontiguousarray(W[:, cv].astype(np.float32))
            bqk_cat = np.concatenate([bb[cq], bb[ck]]).astype(np.float32)
            m[f"bqk{r}"] = np.ascontiguousarray(bqk_cat.reshape(4, 128).T)
            m[f"bv{r}"] = np.ascontiguousarray(
                bb[cv].astype(np.float32).reshape(HG, 64).T)
        in_maps.append(m)
    return in_maps


def assemble(results, bo):
    out = np.empty((B, T, D), np.float32)
    for c in range(8):
        b, g = divmod(c, G)
        out[b, TS * g:TS * (g + 1), :] = results[c]["y"]
    return out + bo.astype(np.float32)


def kernel(x, Wqkv0, bqkv0, Wqkv1, bqkv1, Wg, bg, Wo, bo):
    args = [np.asarray(a) for a in
            (x, Wqkv0, bqkv0, Wqkv1, bqkv1, Wg, bg, Wo, bo)]
    nc = _get_nc()
    in_maps = make_in_maps(*args)
    res = bass_utils.run_bass_kernel_spmd(nc, in_maps, core_ids=list(range(8)))
    return assemble(res.results, args[8])



# revision 12
# speedup vs baseline: 92.7777x; 2.0696x over previous
"""BoostedCausalAttention on 8 trn2 NeuronCores.

Sharding: core c -> (batch b = c//4, head-group g = c%4, 4 heads each).
Within a 4-core batch group (Megatron-style):
  - qkv projections + attention computed per head-group in "transposed"
    layout (feature on partitions, token on free axis); weights and
    activations are bf16 (PE rate is the same as f32r, DMA/collective
    bytes and DVE cycles halve), accumulation stays f32 in PSUM.
  - Per-head-pair AllGathers of pred^T (residual path); corr^T moves by
    per-head-pair AllToAll (cc_dim=Free) since the gate only needs each
    core's own 512-token strip (4x less wire than a full gather). The
    gate MLP + output projection run on that strip; the pred strip is
    selected by a partition_id-derived dynamic DMA offset.
  - Final output: each core emits y[512 tokens, 1024] (pre-bias); host
    concatenates and adds bo.
"""

from contextlib import ExitStack

import numpy as np

import concourse.bass as bass
import concourse.bacc as bacc
import concourse.tile as tile
import concourse.mybir as mybir
from concourse import bass_utils

B, T, D = 2, 2048, 1024
H, DH = 16, 64
SCALE = DH ** -0.5
G = 4            # head groups (cores per batch)
HG = H // G      # heads per core = 4
CP = HG * DH     # channels per core = 256
TS = T // G      # token slice per core for gate/output phase = 512
KC = D // 128    # contraction chunks over D = 8
MASK_VAL = -30000.0

F32 = mybir.dt.float32
F32R = mybir.dt.float32r
BF16 = mybir.dt.bfloat16
ID = mybir.ActivationFunctionType.Identity
EXP = mybir.ActivationFunctionType.Exp
SIG = mybir.ActivationFunctionType.Sigmoid

GROUPS = [[0, 1, 2, 3], [4, 5, 6, 7]]


def _build(sim=False, repeat=1):
    nc = bacc.Bacc("TRN2", target_bir_lowering=False, debug=False, num_devices=8)

    xT = nc.dram_tensor("xT", [D, T], BF16, kind="ExternalInput")
    wqk0 = nc.dram_tensor("wqk0", [D, 2 * CP], BF16, kind="ExternalInput")
    wv0 = nc.dram_tensor("wv0", [D, CP], BF16, kind="ExternalInput")
    bqk0 = nc.dram_tensor("bqk0", [128, 4], F32, kind="ExternalInput")
    bv0 = nc.dram_tensor("bv0", [64, HG], F32, kind="ExternalInput")
    wqk1 = nc.dram_tensor("wqk1", [D, 2 * CP], BF16, kind="ExternalInput")
    wv1 = nc.dram_tensor("wv1", [D, CP], BF16, kind="ExternalInput")
    bqk1 = nc.dram_tensor("bqk1", [128, 4], F32, kind="ExternalInput")
    bv1 = nc.dram_tensor("bv1", [64, HG], F32, kind="ExternalInput")
    wg = nc.dram_tensor("wg", [2 * D, D], BF16, kind="ExternalInput")
    bg = nc.dram_tensor("bg", [128, D // 128], F32, kind="ExternalInput")
    wo = nc.dram_tensor("wo", [D, D], BF16, kind="ExternalInput")
    mask = nc.dram_tensor("mask", [128, 128], F32, kind="ExternalInput")
    onesc = nc.dram_tensor("onesc", [128, HG], F32R, kind="ExternalInput")
    ones64 = nc.dram_tensor("ones64", [1, 64], F32R, kind="ExternalInput")
    y = nc.dram_tensor("y", [TS, D], F32, kind="ExternalOutput")

    with tile.TileContext(nc) as tc:
        for _rep in range(repeat):
            _build_body(nc, tc, sim, _rep,
                        xT, wqk0, wv0, bqk0, bv0, wqk1, wv1, bqk1, bv1,
                        wg, bg, wo, mask, onesc, ones64, y)

    nc.compile()
    return nc


def _build_body(nc, tc, sim, _rep,
                xT, wqk0, wv0, bqk0, bv0, wqk1, wv1, bqk1, bv1,
                wg, bg, wo, mask, onesc, ones64, y):
    with ExitStack() as glb:
        consts = glb.enter_context(tc.tile_pool(name="consts", bufs=1))
        # 8KB-per-partition slots: x^T/residual tiles, later reused by Wg
        big8 = glb.enter_context(tc.tile_pool(name="big8", bufs=8))
        dpool = glb.enter_context(tc.tile_pool(name="dpool", bufs=1, space="DRAM"))
        dsmall = glb.enter_context(tc.tile_pool(name="dsmall", bufs=4, space="DRAM"))

        # ---- constants ----
        bqk_sb = [consts.tile([128, 4], F32, name=f"bqk_sb{r}") for r in range(2)]
        bv_sb = [consts.tile([64, HG], F32, name=f"bv_sb{r}") for r in range(2)]
        bg_sb = consts.tile([128, D // 128], F32)
        mask_sb = consts.tile([128, 128], F32)
        ones_sb = consts.tile([128, HG], F32R)
        ones64_sb = consts.tile([1, 64], F32R)
        nc.gpsimd.dma_start(ones_sb[:], onesc[:, :])
        nc.gpsimd.dma_start(ones64_sb[:], ones64[:, :])
        nc.gpsimd.dma_start(bqk_sb[0][:], bqk0[:, :])
        nc.gpsimd.dma_start(bqk_sb[1][:], bqk1[:, :])
        nc.gpsimd.dma_start(bv_sb[0][:], bv0[:, :])
        nc.gpsimd.dma_start(bv_sb[1][:], bv1[:, :])
        nc.gpsimd.dma_start(bg_sb[:], bg[:, :])
        nc.gpsimd.dma_start(mask_sb[:], mask[:, :])

        # ---- x^T resident (becomes residual in place after round 0) ----
        xt = []
        for kc in range(KC):
            xt.append(big8.tile([128, T], BF16, name=f"xt{kc}", tag="b8"))

        # ---- collective DRAM buffers ----
        pred_part = dpool.tile([CP, T], BF16)          # round-0 AG input
        corr_part = dpool.tile([CP, T], BF16)          # round-1 A2A input
        # pred: AG outputs split by head-pair so each half gathers as soon
        # as that pair's attention finishes: rows = 4 group members x 128.
        pred_fh = [dpool.tile([4 * 128, T], BF16, name=f"pred_fh{i}")
                   for i in range(2)]
        corr_fh = [dpool.tile([4 * 128, T], BF16, name=f"corr_fh{i}")
                   for i in range(2)]

        wqk_pool = glb.enter_context(tc.tile_pool(name="wqk", bufs=KC))
        qk_pool = glb.enter_context(tc.tile_pool(name="qk", bufs=6))
        with ExitStack() as att:
            wv_pool = att.enter_context(tc.tile_pool(name="wv", bufs=KC))
            vaug_pool = att.enter_context(tc.tile_pool(name="vaug", bufs=16))
            p_pool = att.enter_context(tc.tile_pool(name="pp", bufs=3))
            o_pool = att.enter_context(tc.tile_pool(name="op", bufs=4))
            bc_pool = att.enter_context(tc.tile_pool(name="bc", bufs=2))
            sm_pool = att.enter_context(tc.tile_pool(name="sm", bufs=1))
            psum = att.enter_context(tc.tile_pool(name="psum", bufs=2, space="PSUM"))
            avps = att.enter_context(tc.tile_pool(name="avps", bufs=4, space="PSUM"))

            def load_wqk(wqk_d, rnd):
                wt = []
                for kc in range(KC):
                    t_ = wqk_pool.tile([128, 2 * CP], BF16,
                                       name=f"wqk{rnd}_{kc}", tag="wqk")
                    eng = nc.scalar if kc % 2 else nc.sync
                    eng.dma_start(t_[:], wqk_d[128 * kc:128 * (kc + 1), :])
                    wt.append(t_)
                return wt

            def load_wv(wv_d, rnd):
                wt = []
                for kc in range(KC):
                    t_ = wv_pool.tile([128, CP], BF16,
                                      name=f"wv{rnd}_{kc}", tag="wv")
                    eng = nc.scalar if kc % 2 else nc.sync
                    eng.dma_start(t_[:], wv_d[128 * kc:128 * (kc + 1), :])
                    wt.append(t_)
                return wt

            def proj_qk(wt, src, biasc, rnd):
                """q^T|k^T [512 rows, T] as 4 tiles [128, T] (0-1: q, 2-3: k)."""
                qk = []
                for jc in range(4):
                    qk.append(qk_pool.tile([128, T], BF16,
                                           name=f"qk{rnd}_{jc}", tag="qk"))
                for t4 in range(4):
                    for jc in range(4):
                        ps = psum.tile([128, 512], F32, tag="ps", name="ps_pqk")
                        for i, kc in enumerate([0, 2, 4, 6, 1, 3, 5, 7]):
                            nc.tensor.matmul(
                                ps[:], wt[kc][:, 128 * jc:128 * (jc + 1)],
                                src[kc][:, 512 * t4:512 * (t4 + 1)],
                                start=(i == 0), stop=(i == KC - 1))
                        nc.vector.tensor_scalar_add(
                            qk[jc][:, 512 * t4:512 * (t4 + 1)], ps[:],
                            biasc[:, jc:jc + 1])
                return qk

            def proj_v(wt, src, rnd):
                """v in natural layout + ones col: 16 tiles [128, HG, DH+1]."""
                va = []
                for tb in range(16):
                    t_ = vaug_pool.tile([128, HG, DH + 1], BF16,
                                        name=f"va{rnd}_{tb}", tag="va")
                    ps = psum.tile([128, 512], F32, tag="ps", name="ps_pv")
                    for i, kc in enumerate([0, 2, 4, 6, 1, 3, 5, 7]):
                        nc.tensor.matmul(
                            ps[:, 0:CP], src[kc][:, 128 * tb:128 * (tb + 1)],
                            wt[kc][:], start=(i == 0), stop=(i == KC - 1))
                    nc.vector.tensor_copy(
                        t_[:, :, 0:DH],
                        ps[:, 0:CP].rearrange("p (h d) -> p h d", h=HG))
                    nc.vector.tensor_copy(t_[:, :, DH:DH + 1],
                                          ones_sb[:, :, None])
                    va.append(t_)
                return va

            def attend(qk, va, biasv, part_dst, hh_done=None):
                """Causal attention for 4 heads; writes normalized pred^T
                parts straight to DRAM (AG layout and/or A2A layout).

                The two heads of a pair (h2=0,1) share one [128,1024] PSUM
                score tile and a single merged exp, halving Activation-engine
                instruction count (exp is the attention-phase bottleneck)."""
                for hh in range(2):      # head pairs share PE via row groups
                    ob = [o_pool.tile([64, T], BF16, tag="o", name=f"o{h2}")
                          for h2 in range(2)]
                    for q4 in range(4):
                        nblk = 4 * (q4 + 1)
                        av = [avps.tile([DH + 1, 512], F32, tag="av",
                                        name=f"av{h2}") for h2 in range(2)]
                        for kb in range(nblk):
                            diag = kb - 4 * q4
                            c0 = max(0, 128 * diag)
                            npr = 512 - c0
                            qt, kt = qk[hh], qk[2 + hh]
                            ps = psum.tile([128, 1024], F32, tag="ps",
                                           name="s2")
                            for h2 in range(2):
                                base = 64 * h2
                                nc.tensor.matmul(
                                    ps[:, 512 * h2:512 * h2 + npr],
                                    kt[base:base + 64,
                                       128 * kb:128 * (kb + 1)],
                                    qt[base:base + 64,
                                       512 * q4 + c0:512 * (q4 + 1)],
                                    start=True, stop=True)
                            if diag >= 0:
                                nc.vector.tensor_add(
                                    ps[:, 0:128], ps[:, 0:128], mask_sb[:])
                                nc.vector.tensor_add(
                                    ps[:, 512:640], ps[:, 512:640], mask_sb[:])
                            p = p_pool.tile([128, 1024], BF16, tag="p",
                                            name="p2")
                            if npr == 512:
                                nc.scalar.activation(p[:, 0:1024],
                                                     ps[:, 0:1024],
                                                     EXP, scale=SCALE)
                            else:
                                p2v = p[:].rearrange(
                                    "pp (h x) -> pp h x", h=2)
                                ps2v = ps[:].rearrange(
                                    "pp (h x) -> pp h x", h=2)
                                nc.scalar.activation(p2v[:, :, 0:npr],
                                                     ps2v[:, :, 0:npr],
                                                     EXP, scale=SCALE)
                            for h2 in range(2):
                                nc.tensor.matmul(
                                    av[h2][:, c0:512],
                                    va[kb][:, 2 * hh + h2, :],
                                    p[:, 512 * h2:512 * h2 + npr],
                                    start=(kb == 0), stop=(kb == nblk - 1))
                        recr = sm_pool.tile([1, 1024], F32R, tag="recr",
                                            name="recr")
                        with nc.allow_low_precision(
                                reason="softmax recip rounds to f32r"):
                            nc.vector.reciprocal(recr[0:1, 0:512],
                                                 av[0][DH:DH + 1, :])
                            nc.vector.reciprocal(recr[0:1, 512:1024],
                                                 av[1][DH:DH + 1, :])
                        bps = psum.tile([128, 1024], F32, tag="ps",
                                        name="bps")
                        for h2 in range(2):
                            nc.tensor.matmul(
                                bps[0:64, 512 * h2:512 * (h2 + 1)],
                                ones64_sb[:],
                                recr[0:1, 512 * h2:512 * (h2 + 1)],
                                start=True, stop=True)
                        bc = bc_pool.tile([64, 1024], F32, tag="bc",
                                          name="bc")
                        nc.vector.tensor_copy(bc[:], bps[0:64, :])
                        for h2 in range(2):
                            h = 2 * hh + h2
                            osl = ob[h2][:, 512 * q4:512 * (q4 + 1)]
                            nc.vector.tensor_mul(
                                osl, av[h2][0:DH, :],
                                bc[:, 512 * h2:512 * (h2 + 1)])
                            nc.vector.tensor_scalar_add(osl, osl,
                                                        biasv[:, h:h + 1])
                    for h2 in range(2):
                        h = 2 * hh + h2
                        nc.scalar.dma_start(
                            part_dst[64 * h:64 * (h + 1), :], ob[h2][:])
                    if hh_done is not None:
                        hh_done(hh)

            # ================= round 0 =================
            wv0_t = load_wv(wv0, 0)
            for hf in range(4):
                for kc in range(KC):
                    eng = nc.scalar if (kc + hf) % 2 else nc.sync
                    eng.dma_start(xt[kc][:, 512 * hf:512 * (hf + 1)],
                                  xT[128 * kc:128 * (kc + 1),
                                     512 * hf:512 * (hf + 1)])
                if hf == 0:
                    wqk0_t = load_wqk(wqk0, 0)
            va0 = proj_v(wv0_t, xt, 0)
            qk0 = proj_qk(wqk0_t, xt, bqk_sb[0], 0)
            def ag_pred(hh):
                if sim:
                    nc.sync.dma_start(pred_fh[hh][0:128, 0:64],
                                      pred_part[128 * hh:128 * (hh + 1), 0:64])
                else:
                    nc.gpsimd.collective_compute(
                        "AllGather", mybir.AluOpType.bypass,
                        replica_groups=GROUPS,
                        ins=[pred_part[128 * hh:128 * (hh + 1), :]],
                        outs=[pred_fh[hh][:, :]])

            attend(qk0, va0, bv_sb[0], pred_part, hh_done=ag_pred)

            # residual in place: xt <- xt - pred^T. 512-column blocks so the
            # t4-major round-1 projection unblocks per quarter; subs split
            # across DVE and GpSimd so two run concurrently. Even kc chunks
            # (available after the first half-AG) go first.
            pt_pool = att.enter_context(tc.tile_pool(name="pt", bufs=4))
            for qcol in range(4):
                cs = slice(512 * qcol, 512 * (qcol + 1))
                for i, kc in enumerate([0, 2, 4, 6, 1, 3, 5, 7]):
                    pt = pt_pool.tile([128, 512], BF16, tag="pt",
                                      name="predld")
                    src_t = pred_fh[kc % 2]
                    r0 = 128 * (kc // 2)
                    eng_d = nc.sync if i % 2 else nc.scalar
                    eng_d.dma_start(pt[:], src_t[r0:r0 + 128, cs])
                    eng_v = nc.vector if i % 2 else nc.gpsimd
                    eng_v.tensor_sub(xt[kc][:, cs], xt[kc][:, cs], pt[:])

            # ================= round 1 =================
            wv1_t = load_wv(wv1, 1)
            wqk1_t = load_wqk(wqk1, 1)
            va1 = proj_v(wv1_t, xt, 1)
            qk1 = proj_qk(wqk1_t, xt, bqk_sb[1], 1)

            # prefetch gate-phase pred slices into the wqk slots that free
            # up after the round-1 projection (overlaps attention 1)
            pid = nc.scalar.partition_id()
            greg = nc.scalar.alloc_register(f"gslice{_rep}")
            nc.scalar.reg_mod(greg, pid, G)
            gsel = nc.scalar.snap(greg, donate=True, min_val=0, max_val=G - 1)
            pred_vh = [t[:, :].rearrange("d (s t) -> d s t", s=G)
                       for t in pred_fh]
            predg = []
            for cc in range(KC):
                pg_ = wqk_pool.tile([128, TS], BF16, name=f"predg{cc}",
                                    tag="wqk")
                r0 = 128 * (cc // 2)
                nc.scalar.dma_start(
                    pg_[:], pred_vh[cc % 2][r0:r0 + 128,
                                            bass.ds(gsel, 1), :].squeeze(1))
                predg.append(pg_)

            def ag_corr(hh):
                if sim:
                    nc.sync.dma_start(corr_fh[hh][0:128, 0:64],
                                      corr_part[128 * hh:128 * (hh + 1), 0:64])
                else:
                    nc.gpsimd.collective_compute(
                        "AllGather", mybir.AluOpType.bypass,
                        replica_groups=GROUPS,
                        ins=[corr_part[128 * hh:128 * (hh + 1), :]],
                        outs=[corr_fh[hh][:, :]])

            attend(qk1, va1, bv_sb[1], corr_part, hh_done=ag_corr)

        # ================= gate + output ==================
        with ExitStack() as gat:
            gp_pool = gat.enter_context(tc.tile_pool(name="gp", bufs=8))
            wo_pool = gat.enter_context(tc.tile_pool(name="wo", bufs=KC))
            y_pool = gat.enter_context(tc.tile_pool(name="yp", bufs=2))
            ps2 = gat.enter_context(tc.tile_pool(name="ps2", bufs=8,
                                                 space="PSUM"))

            # Wg reuses the 8KB big8 slots freed by x^T/residual:
            # tile i holds contraction chunks 2i (cols 0:1024) and 2i+1.
            wg_t = []
            for i in range(KC):
                t_ = big8.tile([128, 2 * D], BF16, name=f"wg{i}", tag="b8")
                nc.scalar.dma_start(
                    t_[:].rearrange("p (a d) -> p a d", a=2),
                    bass.AP(tensor=wg, offset=256 * i * D,
                            ap=[[D, 128], [128 * D, 2], [1, D]]))
                wg_t.append(t_)

            def wg_slice(cc, jc):
                return wg_t[cc // 2][:, D * (cc % 2) + 128 * jc:
                                     D * (cc % 2) + 128 * (jc + 1)]

            wo_t = []
            for cc in range(KC):
                t_ = wo_pool.tile([128, D], BF16, name=f"wo{cc}", tag="wo")
                nc.sync.dma_start(t_[:], wo[128 * cc:128 * (cc + 1), :])
                wo_t.append(t_)

            corr_vh = [t[:, :].rearrange("d (s t) -> d s t", s=G)
                       for t in corr_fh]
            corr_t = [qk_pool.tile([128, T], BF16, name=f"corrt{i}", tag="qk")
                      for i in range(2)]
            for cc in range(KC):
                r0 = 128 * (cc // 2)
                nc.scalar.dma_start(
                    corr_t[cc // 4][:, 512 * (cc % 4):512 * (cc % 4 + 1)],
                    corr_vh[cc % 2][r0:r0 + 128,
                                    bass.ds(gsel, 1), :].squeeze(1))
            corrg = [corr_t[cc // 4][:, 512 * (cc % 4):512 * (cc % 4 + 1)]
                     for cc in range(KC)]

            gps = []
            for jc in range(KC):
                ps = ps2.tile([128, 512], F32, tag="ps2", name=f"ps_g{jc}")
                for cc in range(KC):
                    nc.tensor.matmul(ps[:], wg_slice(cc, jc), predg[cc][:],
                                     start=(cc == 0), stop=False)
                gps.append(ps)
            pgt = []
            for jc in range(KC):
                ps = gps[jc]
                for cc in range(KC, 16):
                    nc.tensor.matmul(ps[:], wg_slice(cc, jc), corrg[cc - KC],
                                     start=False, stop=(cc == 15))
                gt = gp_pool.tile([128, TS], BF16, name=f"gate{jc}", tag="gp")
                nc.scalar.activation(gt[:], ps[:], SIG, bias=bg_sb[:, jc:jc + 1])
                nc.vector.tensor_mul(gt[:], gt[:], corrg[jc])
                nc.vector.tensor_add(gt[:], gt[:], predg[jc][:])
                pgt.append(gt)

            for tb in range(4):
                yt = y_pool.tile([128, D], F32, tag="y", name="yt")
                for n2 in range(2):
                    ps = ps2.tile([128, 512], F32, tag="ps2", name="ps_y")
                    for cc in range(KC):
                        nc.tensor.matmul(
                            ps[:], pgt[cc][:, 128 * tb:128 * (tb + 1)],
                            wo_t[cc][:, 512 * n2:512 * (n2 + 1)],
                            start=(cc == 0), stop=(cc == KC - 1))
                    nc.vector.tensor_copy(yt[:, 512 * n2:512 * (n2 + 1)],
                                          ps[:])
                nc.sync.dma_start(y[128 * tb:128 * (tb + 1), :], yt[:])


_NC = None


def _get_nc():
    global _NC
    if _NC is None:
        _NC = _build()
    return _NC


def make_in_maps(x, Wqkv0, bqkv0, Wqkv1, bqkv1, Wg, bg, Wo, bo):
    import ml_dtypes
    bf16 = np.dtype(ml_dtypes.bfloat16)
    mask_np = np.where(np.arange(128)[:, None] > np.arange(128)[None, :],
                       np.float32(MASK_VAL), np.float32(0.0)).astype(np.float32)
    ones_np = np.ones((128, HG), np.float32)
    bg_a = np.ascontiguousarray(bg.reshape(D // 128, 128).T.astype(np.float32))
    wg_np = np.ascontiguousarray(Wg.astype(bf16))
    wo_np = np.ascontiguousarray(Wo.astype(bf16))

    in_maps = []
    for c in range(8):
        b, g = divmod(c, G)
        cq = slice(CP * g, CP * (g + 1))
        ck = slice(D + CP * g, D + CP * (g + 1))
        cv = slice(2 * D + CP * g, 2 * D + CP * (g + 1))
        m = {
            "xT": np.ascontiguousarray(x[b].T.astype(bf16)),
            "mask": mask_np, "onesc": ones_np, "bg": bg_a,
            "ones64": np.ones((1, 64), np.float32),
            "wg": wg_np, "wo": wo_np,
        }
        for r, (W, bb) in enumerate(((Wqkv0, bqkv0), (Wqkv1, bqkv1))):
            m[f"wqk{r}"] = np.ascontiguousarray(
                np.concatenate([W[:, cq], W[:, ck]], axis=1).astype(bf16))
            m[f"wv{r}"] = np.ascontiguousarray(W[:, cv].astype(bf16))
            bqk_cat = np.concatenate([bb[cq], bb[ck]]).astype(np.float32)
            m[f"bqk{r}"] = np.ascontiguousarray(bqk_cat.reshape(4, 128).T)
            m[f"bv{r}"] = np.ascontiguousarray(
                bb[cv].astype(np.float32).reshape(HG, 64).T)
        in_maps.append(m)
    return in_maps


def assemble(results, bo):
    out = np.empty((B, T, D), np.float32)
    for c in range(8):
        b, g = divmod(c, G)
        out[b, TS * g:TS * (g + 1), :] = results[c]["y"]
    return out + bo.astype(np.float32)


def kernel(x, Wqkv0, bqkv0, Wqkv1, bqkv1, Wg, bg, Wo, bo):
    args = [np.asarray(a) for a in
            (x, Wqkv0, bqkv0, Wqkv1, bqkv1, Wg, bg, Wo, bo)]
    nc = _get_nc()
    in_maps = make_in_maps(*args)
    res = bass_utils.run_bass_kernel_spmd(nc, in_maps, core_ids=list(range(8)))
    return assemble(res.results, args[8])

